# revision 6
# baseline (speedup 1.0000x reference)
"""GAT layer (nn_ClusteringModel) Trainium2 kernel, 8-core SPMD.

Math: for each head h,
  proj = x @ P_h                                  [N, 64]
  s_src = proj @ q_src_h,  s_trg = proj @ q_trg_h [N]
  att = softmax_j(leaky_relu(s_src_i + s_trg_j, .2) + mask_ij)
  out_h = att @ proj ; out = elu(concat_h(out_h) + x @ Wskip^T + bias)

Key identity used on-device (no transcendentals on the [j,i] tiles):
  exp(leaky(s_i+t_j)) = e^{.2 s_i} * max(a_i*v_j, v2_j)
      a = e^{.8 s_src}, v = e^{s_trg}, v2 = e^{.2 s_trg}
and the e^{.2 s_i} factor cancels in the softmax ratio. With adj01 = (mask==0):
  att[i,j] = E[j,i]/D[i],  E[j,i] = (relu(v_j*a_i - v2_j) + v2_j)*adj01[j,i]
So per 128x512 tile: one ACT Relu (per-partition scale/bias) + one fused DVE
scalar_tensor_tensor, then the aggregation matmul accumulates
[proj | ones]^T @ E into PSUM, giving both att@proj and the denominator D.

Sharding: rows (target nodes i) split 512/core across 8 cores; projected
features computed per-core for its own j-rows and AllGathered.
"""

import sys

for _p in ("/opt/trn_rl_repo",):
    if _p not in sys.path:
        sys.path.insert(0, _p)

import copy
import json

import ml_dtypes
import numpy as np

import concourse.bass as bass
import concourse.mybir as mybir
import concourse.tile as tile
from concourse.bass_utils import run_bass_kernel_spmd
from concourse.masks import make_identity

# ---------------------------------------------------------------------------
# Workaround for this container's walrus build: CTRL-class instructions
# (Drain/EventSemaphore/Nop) accept only ONE sem-wait command; Tile's
# end-of-context drain aggregates more. Split excess waits onto preceding
# single-wait Drains on the same engine (same-engine program order makes this
# semantically identical).
_MAX_WAITS = 1
_SKIP_OPS = {"UnconditionalBranch", "Call", "RegisterMove"}


def _split_waits(bir: dict) -> dict:
    for f in bir.get("functions", []):
        for b in f.get("blocks", []):
            insts = b.get("instructions") or []
            out = []
            for inst in insts:
                si = inst.get("sync_info") or {}
                ow = si.get("on_wait") or []
                if len(ow) > _MAX_WAITS and inst.get("opcode") not in _SKIP_OPS:
                    keep = ow[-_MAX_WAITS:]
                    extra = ow[:-_MAX_WAITS]
                    for k, w in enumerate(extra):
                        out.append({
                            "name": f"{inst['name']}-ws{k}",
                            "opcode": "NoOp",
                            "engine": inst.get("engine", "SP"),
                            "ins": [],
                            "outs": [],
                            "sync_info": {"on_update": [], "on_wait": [w]},
                            "debug": inst.get("debug", 0),
                        })
                    si["on_wait"] = keep
                    inst["sync_info"] = si
                out.append(inst)
            b["instructions"] = out
    return bir


class PatchedBass(bass.Bass):
    def to_json_bytes(self) -> bytes:
        return json.dumps(_split_waits(json.loads(super().to_json_bytes()))).encode()


# ---------------------------------------------------------------------------
N, H, F_IN, F_OUT = 4096, 8, 256, 64
NCORES = 8
P = 128
NI = N // NCORES          # rows (targets) per core = 512
JBLK = N // P             # 32 j-blocks of 128
FB = F_IN // P            # 2 f-blocks
AGC = F_OUT + 1           # per-head AG cols: 64 proj + s_trg
AGW = H * AGC             # 520
CH = H * F_OUT            # 512 output channels
ISUB = NI // P            # 4 i-subblocks per core

F32 = mybir.dt.float32
BF16 = mybir.dt.bfloat16
ALU = mybir.AluOpType
ACTF = mybir.ActivationFunctionType


def build_program():
    nc = PatchedBass(num_devices=NCORES)

    xT = nc.dram_tensor("xT", [F_IN, NI], F32, kind="ExternalInput")
    adjT = nc.dram_tensor("adjT", [N, NI], BF16, kind="ExternalInput")
    p_aug = nc.dram_tensor("p_aug", [H, F_IN, AGC], F32, kind="ExternalInput")
    q_src = nc.dram_tensor("q_src", [F_IN, H], F32, kind="ExternalInput")
    wskipT = nc.dram_tensor("wskipT", [F_IN, CH], F32, kind="ExternalInput")
    bias_row = nc.dram_tensor("bias_row", [1, CH], F32, kind="ExternalInput")
    out = nc.dram_tensor("out", [NI, CH], F32, kind="ExternalOutput")

    ag_in = nc.dram_tensor("ag_in", [NI, AGW], F32)
    ag_out = nc.dram_tensor("ag_out", [N, AGW], F32, addr_space="Shared")

    with tile.TileContext(nc) as tc:
        with tc.tile_pool(name="const", bufs=1) as cp:
            ident = cp.tile([P, P], F32, tag="ident")
            make_identity(nc, ident)
            ones_row = cp.tile([1, P], F32, tag="ones_row")
            nc.gpsimd.memset(ones_row[:], 1.0)
            bias_sb = cp.tile([1, CH], F32, tag="bias_sb")
            nc.sync.dma_start(bias_sb[:], bias_row[:, :])

            xt = []
            wsk = []
            for fb in range(FB):
                t = cp.tile([P, NI], F32, tag=f"xt{fb}")
                nc.sync.dma_start(t[:], xT[fb * P:(fb + 1) * P, :])
                xt.append(t)
                w = cp.tile([P, CH], F32, tag=f"wsk{fb}")
                nc.sync.dma_start(w[:], wskipT[fb * P:(fb + 1) * P, :])
                wsk.append(w)

            pa = [[None] * FB for _ in range(H)]
            for h in range(H):
                for fb in range(FB):
                    t = cp.tile([P, AGC], F32, tag=f"pa{h}_{fb}")
                    nc.sync.dma_start(t[:], p_aug[h, fb * P:(fb + 1) * P, :])
                    pa[h][fb] = t
            qs = cp.tile([P, FB, H], F32, tag="qs")
            nc.sync.dma_start(qs[:], q_src.rearrange("(fb p) h -> p fb h", p=P))

            a_sb = [cp.tile([P, NI], F32, tag=f"a{h}", name=f"a{h}") for h in range(H)]
            v_sb = [cp.tile([P, JBLK], F32, tag=f"v{h}", name=f"v{h}") for h in range(H)]
            v2_sb = [cp.tile([P, JBLK], F32, tag=f"v2{h}", name=f"v2{h}") for h in range(H)]
            nv2_sb = [cp.tile([P, JBLK], F32, tag=f"nv2{h}", name=f"nv2{h}") for h in range(H)]

            # ---------------- phase 1: proj(+s_trg) of own rows, AG, a/v prep
            with tc.tile_pool(name="pp1", bufs=1, space="PSUM") as pp1, \
                 tc.tile_pool(name="w1", bufs=2) as w1:
                for jsub in range(ISUB):
                    agsb = w1.tile([P, AGW], F32, tag=f"agsb")
                    for h in range(H):
                        ps = pp1.tile([P, AGC], F32, tag="ps", bufs=4)
                        for fb in range(FB):
                            nc.tensor.matmul(
                                ps, lhsT=xt[fb][:, jsub * P:(jsub + 1) * P],
                                rhs=pa[h][fb], start=(fb == 0), stop=(fb == FB - 1))
                        nc.vector.tensor_copy(agsb[:, h * AGC:(h + 1) * AGC], ps)
                    nc.sync.dma_start(ag_in[jsub * P:(jsub + 1) * P, :], agsb[:])

                # s_src -> a broadcast
                for h in range(H):
                    psr = pp1.tile([1, NI], F32, tag="psrow", bufs=2)
                    for fb in range(FB):
                        nc.tensor.matmul(psr, lhsT=qs[:, fb, h:h + 1], rhs=xt[fb],
                                         start=(fb == 0), stop=(fb == FB - 1))
                    arow = w1.tile([1, NI], F32, tag="arow")
                    nc.scalar.activation(arow, psr, ACTF.Exp, scale=0.8)
                    abps = pp1.tile([P, NI], F32, tag="abps", bufs=2)
                    nc.tensor.matmul(abps, lhsT=ones_row[:], rhs=arow[:],
                                     start=True, stop=True)
                    nc.vector.tensor_copy(a_sb[h], abps)

                nc.gpsimd.collective_compute(
                    "AllGather", ALU.bypass,
                    replica_groups=[list(range(NCORES))],
                    ins=[ag_in[:, :]], outs=[ag_out[:, :]])

                ag3 = ag_out.rearrange("(b p) c -> p b c", p=P)
                for h in range(H):
                    vraw = w1.tile([P, JBLK], F32, tag="vraw")
                    nc.sync.dma_start(vraw[:], ag3[:, :, h * AGC + F_OUT])
                    nc.scalar.activation(v_sb[h], vraw, ACTF.Exp)
                    nc.scalar.activation(v2_sb[h], vraw, ACTF.Exp, scale=0.2)
                    nc.vector.tensor_scalar(nv2_sb[h], v2_sb[h], -1.0, None, ALU.mult)

            # ---------------- phase 2: fused attention + aggregation
            with tc.tile_pool(name="accsb", bufs=1) as accsbp:
                with tc.tile_pool(name="accp", bufs=1, space="PSUM") as accp, \
                     tc.tile_pool(name="adjp", bufs=3) as adjp, \
                     tc.tile_pool(name="lhp", bufs=16) as lhp, \
                     tc.tile_pool(name="rp", bufs=4) as rp, \
                     tc.tile_pool(name="ep", bufs=4) as ep:
                    accs = [accp.tile([AGC, NI], F32, tag=f"acc{h}", name=f"acc{h}") for h in range(H)]
                    for jb in range(JBLK):
                        adj_t = adjp.tile([P, NI], BF16, tag="adj")
                        nc.sync.dma_start(adj_t[:], adjT[jb * P:(jb + 1) * P, :])
                        for h in range(H):
                            lh = lhp.tile([P, AGC], F32, tag="lh")
                            nc.sync.dma_start(
                                lh[:, 0:F_OUT],
                                ag_out[jb * P:(jb + 1) * P, h * AGC:h * AGC + F_OUT])
                            nc.gpsimd.memset(lh[:, F_OUT:AGC], 1.0)
                            r = rp.tile([P, NI], F32, tag="r")
                            nc.scalar.activation(
                                r, a_sb[h], ACTF.Relu,
                                bias=nv2_sb[h][:, jb:jb + 1],
                                scale=v_sb[h][:, jb:jb + 1])
                            e = ep.tile([P, NI], F32, tag="e")
                            nc.vector.scalar_tensor_tensor(
                                e, r, v2_sb[h][:, jb:jb + 1], adj_t,
                                op0=ALU.add, op1=ALU.mult)
                            nc.tensor.matmul(accs[h], lhsT=lh, rhs=e,
                                             start=(jb == 0), stop=(jb == JBLK - 1))

                    # drain accumulators to SBUF (inside accp scope)
                    acc_sb = [accsbp.tile([AGC, NI], F32, tag=f"accsb{h}", name=f"accsb{h}") for h in range(H)]
                    for h in range(H):
                        nc.vector.tensor_copy(acc_sb[h], accs[h])

                # ---------------- phase 3: normalize, transpose, skip, elu
                with tc.tile_pool(name="pp3", bufs=2, space="PSUM") as pp3, \
                     tc.tile_pool(name="w3", bufs=2) as w3, \
                     tc.tile_pool(name="w3s", bufs=4) as w3s:
                    outsb = [w3.tile([P, CH], F32, tag=f"osb{c}", name=f"osb{c}") for c in range(ISUB)]
                    for h in range(H):
                        for c in range(ISUB):
                            tps = pp3.tile([P, AGC], F32, tag="tps")
                            nc.tensor.transpose(
                                tps, acc_sb[h][:, c * P:(c + 1) * P],
                                ident[0:AGC, 0:AGC])
                            rec = w3s.tile([P, 1], F32, tag="rec")
                            nc.vector.reciprocal(rec, tps[:, F_OUT:AGC])
                            nc.vector.tensor_scalar(
                                outsb[c][:, h * F_OUT:(h + 1) * F_OUT],
                                tps[:, 0:F_OUT], rec, None, ALU.mult)
                    for c in range(ISUB):
                        skp = pp3.tile([P, CH], F32, tag="skp")
                        for fb in range(FB):
                            nc.tensor.matmul(
                                skp, lhsT=xt[fb][:, c * P:(c + 1) * P],
                                rhs=wsk[fb], start=(fb == 0), stop=False)
                        nc.tensor.matmul(skp, lhsT=ones_row[:], rhs=bias_sb[:],
                                         start=False, stop=True)
                        y = w3.tile([P, CH], F32, tag="y")
                        nc.vector.tensor_tensor(y, outsb[c], skp, ALU.add)
                        pos = w3.tile([P, CH], F32, tag="pos")
                        nc.vector.tensor_scalar(pos, y, 0.0, None, ALU.max)
                        neg = w3.tile([P, CH], F32, tag="neg")
                        nc.vector.tensor_scalar(neg, y, 0.0, None, ALU.min)
                        en = w3.tile([P, CH], F32, tag="en")
                        nc.scalar.activation(en, neg, ACTF.Exp)
                        fin = w3.tile([P, CH], F32, tag="fin")
                        nc.vector.scalar_tensor_tensor(
                            fin, en, -1.0, pos, op0=ALU.add, op1=ALU.add)
                        nc.sync.dma_start(out[c * P:(c + 1) * P, :], fin[:])
    return nc


LAST_RESULT = None
_PROG_CACHE = {}


def _get_program():
    if "nc" not in _PROG_CACHE:
        _PROG_CACHE["nc"] = build_program()
    return _PROG_CACHE["nc"]


def kernel(x, connectivity_mask, proj_param, scoring_fn_source,
           scoring_fn_target, skip_proj_w, bias):
    x = np.asarray(x, dtype=np.float32)
    connectivity_mask = np.asarray(connectivity_mask, dtype=np.float32)
    proj_param = np.asarray(proj_param, dtype=np.float32)
    scoring_fn_source = np.asarray(scoring_fn_source, dtype=np.float32)
    scoring_fn_target = np.asarray(scoring_fn_target, dtype=np.float32)
    skip_proj_w = np.asarray(skip_proj_w, dtype=np.float32)
    bias = np.asarray(bias, dtype=np.float32)

    xT_full = np.ascontiguousarray(x.T)                       # [256, 4096]
    adjT_full = np.ascontiguousarray(
        (connectivity_mask == 0.0).T.astype(ml_dtypes.bfloat16))  # [j, i]
    q_trg = np.einsum("hfo,hoe->hfe", proj_param, scoring_fn_target)  # [H,256,1]
    q_src_full = np.einsum("hfo,hoe->hfe", proj_param,
                           scoring_fn_source)[:, :, 0].T.copy()       # [256, H]
    p_aug_full = np.concatenate([proj_param, q_trg], axis=2).astype(np.float32)
    wskipT_full = np.ascontiguousarray(skip_proj_w.T)          # [256, 512]
    bias_row = bias[None, :].astype(np.float32)

    in_maps = []
    for c in range(NCORES):
        sl = slice(c * NI, (c + 1) * NI)
        in_maps.append({
            "xT": np.ascontiguousarray(xT_full[:, sl]),
            "adjT": np.ascontiguousarray(adjT_full[:, sl]),
            "p_aug": p_aug_full,
            "q_src": q_src_full,
            "wskipT": wskipT_full,
            "bias_row": bias_row,
        })

    nc = _get_program()
    res = run_bass_kernel_spmd(nc, in_maps, core_ids=list(range(NCORES)))
    global LAST_RESULT
    LAST_RESULT = res
    return np.concatenate([res.results[c]["out"] for c in range(NCORES)], axis=0)
